# revision 7
# baseline (speedup 1.0000x reference)
"""DynamicConvolution Trainium2 kernel (8 NeuronCores, data-parallel over batch).

Reference computation (per sample b):
  pooled = mean(x[b], spatial)                    [256]
  h      = relu(pooled @ w1 + b1)                 [64]
  s      = h @ w2 + b2                            [8]
  alpha  = softmax(s)                             [8]
  W[b]   = sum_k alpha[k] * kernels[k]            [256,256,3,3]
  y[b]   = conv2d(x[b], W[b], pad=1)              [256,56,56]

Sharding: batch 16 -> 2 samples per core, kernel bank + MLP weights replicated.

Device mapping (per core):
  - x staged in SBUF as zero-padded [128i, 58*58] bf16 images (2 samples x 2
    channel-tiles), written by pad-only memsets (GpSimd) + strided DMAs.
  - pooled via chunked free-dim reduces on VectorE.
  - attention MLP on TensorE with channels on partitions (no transposes);
    bias terms enter as K=1 rank-1 matmuls; softmax on VectorE/ScalarE.
  - alphas flattened [2,8]->[1,16] by DMA, broadcast to [128,16] by a
    K=1 ones-matmul.
  - kernel mixing: per (sample, o_tile, i_tile) a chain of 8 fused
    scalar_tensor_tensor MACs on VectorE over the bf16 bank (fp32 scalar AP).
  - conv: per (o_tile, sample, row-block) PSUM group of 18 accumulating bf16
    matmuls [o128,448] += W[i128,o128]^T @ xpad[i128, 8x56 shifted window],
    over 2 i_tiles x 9 taps.  ScalarE evacuates PSUM -> SBUF fp32; DMA out.

Sync discipline (walrus permits ONE semaphore wait per engine instruction):
  - matmul waits split naturally: lhsT dep on InstLdweights, rhs dep on
    InstMatmult; PSUM-reuse WAR goes on InstMatmult (evac on ScalarE, whose
    sem also covers it).
  - 1x1 "touch" matmuls make TensorE observe each x-chunk DMA queue before
    conv; tiny VectorE copies do the same for bank slices before each MAC.
  - add_dep_helper(sync=False) edges pin the scheduling order of touches.
"""

import numpy as np
import ml_dtypes
from contextlib import ExitStack

try:
    import concourse.bass as bass
except ImportError:  # fresh grading dir: repo paths not on sys.path yet
    import sys
    for p in ("/opt/trn_rl_repo", "/root/.axon_site/_ro/trn_rl_repo"):
        if p not in sys.path:
            sys.path.append(p)
    import concourse.bass as bass

import concourse.mybir as mybir
import concourse.tile as tile
from concourse import bacc
from concourse.tile import add_dep_helper
from concourse.bass_utils import run_bass_kernel_spmd

F32 = mybir.dt.float32
BF16 = mybir.dt.bfloat16
AX = mybir.AxisListType.X
RELU = mybir.ActivationFunctionType.Relu
EXP = mybir.ActivationFunctionType.Exp
COPY = mybir.ActivationFunctionType.Copy
MULT = mybir.AluOpType.mult
ADD = mybir.AluOpType.add

N_CORES = 8
B = 2              # samples per core
C = 256            # channels
IT = 2             # 128-channel i tiles
OT = 2             # 128-channel o tiles
H = W_IMG = 56
HW = H * W_IMG     # 3136
PADW = 58
PADHW = PADW * PADW  # 3364
NT = 7             # row blocks per image
TB = 448           # 8 rows x 56 cols per psum block
S = 9              # conv taps
WSL = S * 128      # 1152 elems per (b,o_t,i_t) weight slice / bank slice
XCH = 4            # x DMA chunks per (b, i_tile)
XROWS = 14         # rows per x chunk

# consts layout (fp32 [128, 336])
C_W1A, C_W1B, C_W2, C_B1, C_B2, C_ONES = 0, 64, 128, 136, 200, 208
C_COLS = 336

_cached = None


def _build(variant="full"):
    nc = bacc.Bacc()
    xin = nc.declare_dram_parameter("x", [B, C, HW], BF16, isOutput=False)
    bankin = nc.declare_dram_parameter("bank", [128, OT * IT * 8 * WSL], BF16,
                                       isOutput=False)
    cin = nc.declare_dram_parameter("consts", [128, C_COLS], F32, isOutput=False)
    y = nc.declare_dram_parameter("y", [B, C, HW], F32, isOutput=True)

    with tile.TileContext(nc) as tc, ExitStack() as ctx:
        sb = ctx.enter_context(tc.tile_pool(name="sb", bufs=1))
        conv_ps = ctx.enter_context(tc.tile_pool(name="cps", bufs=4, space="PSUM"))
        mlp_ps = ctx.enter_context(tc.tile_pool(name="mps", bufs=1, space="PSUM"))
        bc_ps = ctx.enter_context(tc.tile_pool(name="bps", bufs=1, space="PSUM"))
        scr_ps = ctx.enter_context(tc.tile_pool(name="sps", bufs=1, space="PSUM"))

        xpad = sb.tile([128, B * IT * PADHW], BF16, tag="xpad")
        bank = sb.tile([128, OT * IT * 8 * WSL], BF16, tag="bank")
        wsb = sb.tile([128, B * OT * IT * WSL], BF16, tag="wsb")
        outsb = sb.tile([128, OT * B * NT * TB], F32, tag="outsb")
        consts = sb.tile([128, C_COLS], F32, tag="consts")
        scratch = scr_ps.tile([1, 1], F32)

        def xv(b, it):
            """padded image view [128, 58, 58] for sample b, channel tile it"""
            base = (b * IT + it) * PADHW
            return xpad[:, base:base + PADHW].rearrange("p (r c) -> p r c", c=PADW)

        # ---------- 1. pad memsets (GpSimd); mid-cols of (b1,i1) LAST
        memsets = []
        if variant in ("full", "fullmemset", "safe"):
            for b in range(B):
                for it in range(IT):
                    base = (b * IT + it) * PADHW
                    memsets.append(nc.gpsimd.memset(
                        xpad[:, base:base + PADHW], 0.0))
        else:
            for b in range(B):
                for it in range(IT):
                    v = xv(b, it)
                    memsets.append(nc.gpsimd.memset(v[:, 0:1, :], 0.0))   # top
                    memsets.append(nc.gpsimd.memset(v[:, 57:58, :], 0.0)) # bottom
            for b in range(B):
                for it in range(IT):
                    base = (b * IT + it) * PADHW
                    mid = xpad[:, base + 57: base + 57 + 57 * PADW].rearrange(
                        "p (r c) -> p r c", c=PADW)[:, :, 0:2]            # [128,57,2]
                    memsets.append(nc.gpsimd.memset(mid[:], 0.0))

        # ---------- 2. DMAs: consts, x (16 chunks), bank (32 slices)
        nc.sync.dma_start(consts[:], cin[:])
        for b in range(B):
            for it in range(IT):
                v = xv(b, it)
                for cch in range(XCH):
                    r0 = cch * XROWS
                    nc.sync.dma_start(
                        v[:, r0 + 1: r0 + 1 + XROWS, 1:57],
                        xin[b, it * 128:(it + 1) * 128,
                            r0 * W_IMG:(r0 + XROWS) * W_IMG].rearrange(
                                "p (r c) -> p r c", c=W_IMG))
        for ot in range(OT):
            for it in range(IT):
                for k in range(8):
                    off = ((ot * IT + it) * 8 + k) * WSL
                    nc.sync.dma_start(bank[:, off:off + WSL],
                                      bankin[:, off:off + WSL])

        # ---------- 3. PE touches: pool memsets + x chunks
        def pe_touch(ap):
            return nc.tensor.matmul(scratch[:], ap, ap, start=True, stop=True,
                                    skip_group_check=True)

        touches = []
        last_base = (1 * IT + 1) * PADHW            # (b1,i1) mid memset region
        touches.append(pe_touch(xpad[0:1, last_base + 57:last_base + 58]))
        for b in range(B):
            for it in range(IT):
                base = (b * IT + it) * PADHW
                for cch in range(XCH):
                    r0 = cch * XROWS
                    off = base + (r0 + 2) * PADW + 2   # interior of chunk
                    touches.append(pe_touch(xpad[0:1, off:off + 1]))
        for t1, t0 in zip(touches[1:], touches[:-1]):
            add_dep_helper(t1.ins, t0.ins, sync=False, reason="touch chain")

        # ---------- 4. pooled means on DVE (chunked, then combine)
        partials = sb.tile([128, B * IT * XCH], F32, tag="partials")
        pooled = sb.tile([128, IT * B], F32, tag="pooled")  # cols (it, b)
        for b in range(B):
            for it in range(IT):
                v = xv(b, it)
                for cch in range(XCH):
                    r0 = cch * XROWS
                    nc.vector.reduce_sum(
                        partials[:, (b * IT + it) * XCH + cch:
                                 (b * IT + it) * XCH + cch + 1],
                        v[:, r0 + 1: r0 + 1 + XROWS, 1:57],
                        axis=mybir.AxisListType.XY)
        psum2 = sb.tile([128, B * IT], F32, tag="psum2")
        for b in range(B):
            for it in range(IT):
                j = b * IT + it
                nc.vector.reduce_sum(psum2[:, j:j + 1],
                                     partials[:, j * XCH:(j + 1) * XCH], axis=AX)
                nc.vector.tensor_scalar_mul(pooled[:, it * B + b: it * B + b + 1],
                                            psum2[:, j:j + 1], 1.0 / HW)

        # ---------- 5. attention MLP on PE
        hT_ps = mlp_ps.tile([64, B], F32)
        nc.tensor.matmul(hT_ps[:], consts[0:1, C_B1:C_B1 + 64],
                         consts[0:1, C_ONES:C_ONES + B], start=True, stop=False)
        nc.tensor.matmul(hT_ps[:], consts[:, C_W1A:C_W1A + 64],
                         pooled[:, 0:B], start=False, stop=False)
        nc.tensor.matmul(hT_ps[:], consts[:, C_W1B:C_W1B + 64],
                         pooled[:, B:2 * B], start=False, stop=True)
        hT = sb.tile([64, B], F32, tag="hT")
        nc.scalar.activation(hT[:], hT_ps[:], RELU)

        sc_ps = mlp_ps.tile([B, 8], F32)
        nc.tensor.matmul(sc_ps[:], consts[0:1, C_ONES:C_ONES + B],
                         consts[0:1, C_B2:C_B2 + 8], start=True, stop=False)
        nc.tensor.matmul(sc_ps[:], hT[:], consts[0:64, C_W2:C_W2 + 8],
                         start=False, stop=True)

        # ---------- 6. softmax (DVE/ACT), flatten, broadcast
        scores = sb.tile([B, 8], F32, tag="scores")
        nc.vector.tensor_copy(scores[:], sc_ps[:])
        mx = sb.tile([B, 1], F32, tag="mx")
        nc.vector.reduce_max(mx[:], scores[:], axis=AX)
        subb = sb.tile([B, 8], F32, tag="subb")
        nc.vector.tensor_scalar_sub(subb[:], scores[:], mx[:])
        ex = sb.tile([B, 8], F32, tag="ex")
        nc.scalar.activation(ex[:], subb[:], EXP)
        z = sb.tile([B, 1], F32, tag="z")
        nc.vector.reduce_sum(z[:], ex[:], axis=AX)
        rz = sb.tile([B, 1], F32, tag="rz")
        nc.vector.reciprocal(rz[:], z[:])
        al = sb.tile([B, 8], F32, tag="al")
        nc.vector.tensor_scalar_mul(al[:], ex[:], rz[:])

        al_flat = sb.tile([1, B * 8], F32, tag="al_flat")
        nc.sync.dma_start(al_flat[:], al[:])
        abc_ps = bc_ps.tile([128, B * 8], F32)
        nc.tensor.matmul(abc_ps[:], consts[0:1, C_ONES:C_ONES + 128],
                         al_flat[:], start=True, stop=True)
        abc = sb.tile([128, B * 8], F32, tag="abc")
        al_evac = nc.vector.tensor_copy(abc[:], abc_ps[:])

        # ---------- 7. kernel mixing on DVE (bf16 MAC chains, fp32 scalar)
        btch = sb.tile([1, 1], BF16, tag="btch")
        first_bank_touch = None
        for ot in range(OT):
            for it in range(IT):
                for k in range(8):
                    boff = ((ot * IT + it) * 8 + k) * WSL
                    tch = nc.vector.tensor_copy(btch[:], bank[0:1, boff:boff + 1])
                    if first_bank_touch is None:
                        first_bank_touch = tch
                        add_dep_helper(tch.ins, al_evac.ins, sync=False,
                                       reason="mixing after attention on DVE")
                    for b in range(B):
                        woff = ((b * OT + ot) * IT + it) * WSL
                        wslice = wsb[:, woff:woff + WSL]
                        acol = abc[:, b * 8 + k: b * 8 + k + 1]
                        bslice = bank[:, boff:boff + WSL]
                        if k == 0:
                            op = nc.vector.tensor_scalar_mul(wslice, bslice, acol)
                        else:
                            op = nc.vector.scalar_tensor_tensor(
                                wslice, bslice, acol, wslice, op0=MULT, op1=ADD)
                        add_dep_helper(op.ins, tch.ins, sync=False,
                                       reason="bank slice observed")

        # ---------- 8. conv: (o_t, b, t) groups of 18 accumulating matmuls
        if variant == "noconv":
            for ot in range(OT):
                for b in range(B):
                    for t in range(NT):
                        blk = (ot * B + b) * NT + t
                        nc.sync.dma_start(
                            y[b, ot * 128:(ot + 1) * 128, t * TB:(t + 1) * TB],
                            outsb[:, blk * TB:(blk + 1) * TB])
            nc.compile()
            return nc
        first_conv = None
        for ot in range(OT):
            for b in range(B):
                for t in range(NT):
                    if variant == "conv1" and (ot, b, t) != (0, 0, 0):
                        continue
                    ps = conv_ps.tile([128, TB], F32, tag="convps")
                    n_mm = 0
                    for it in range(IT):
                        woff = ((b * OT + ot) * IT + it) * WSL
                        v = xv(b, it)
                        for s in range(S):
                            kh, kw = s // 3, s % 3
                            mm = nc.tensor.matmul(
                                ps[:],
                                wsb[:, woff + s * 128: woff + (s + 1) * 128],
                                v[:, 8 * t + kh: 8 * t + kh + 8, kw:kw + 56],
                                start=(n_mm == 0), stop=(n_mm == 17))
                            if first_conv is None:
                                first_conv = mm
                                add_dep_helper(mm.ins, touches[-1].ins,
                                               sync=False, reason="x observed")
                            n_mm += 1
                    blk = (ot * B + b) * NT + t
                    if variant == "conv1" and blk > 0:
                        continue
                    if variant in ("dve_evac", "safe"):
                        nc.vector.tensor_copy(
                            outsb[:, blk * TB:(blk + 1) * TB], ps[:])
                    else:
                        nc.scalar.activation(outsb[:, blk * TB:(blk + 1) * TB],
                                             ps[:], COPY)
                    nc.sync.dma_start(
                        y[b, ot * 128:(ot + 1) * 128, t * TB:(t + 1) * TB],
                        outsb[:, blk * TB:(blk + 1) * TB])

    nc.compile()
    return nc


def _prep(x, kernels, w1, b1, w2, b2):
    """Host-side marshaling: dtype casts + layout rearrangement only."""
    xh = np.ascontiguousarray(
        x.reshape(16, C, HW).astype(ml_dtypes.bfloat16))
    # bank[i_lo, (o_t, i_t, k, s, o_lo)]
    kr = kernels.reshape(8, OT, 128, IT, 128, S)          # k,o_t,o_lo,i_t,i_lo,s
    bankh = np.ascontiguousarray(
        kr.transpose(4, 1, 3, 0, 5, 2).reshape(128, OT * IT * 8 * WSL)
        .astype(ml_dtypes.bfloat16))
    consts = np.zeros((128, C_COLS), dtype=np.float32)
    consts[:, C_W1A:C_W1A + 64] = w1[0:128]
    consts[:, C_W1B:C_W1B + 64] = w1[128:256]
    consts[0:64, C_W2:C_W2 + 8] = w2
    consts[0, C_B1:C_B1 + 64] = b1
    consts[0, C_B2:C_B2 + 8] = b2
    consts[0, C_ONES:C_ONES + 128] = 1.0
    return xh, bankh, consts


def kernel(x, kernels, w1, b1, w2, b2):
    global _cached
    if _cached is None:
        _cached = _build()
    nc = _cached
    xh, bankh, consts = _prep(np.asarray(x, dtype=np.float32),
                              np.asarray(kernels, dtype=np.float32),
                              np.asarray(w1, dtype=np.float32),
                              np.asarray(b1, dtype=np.float32),
                              np.asarray(w2, dtype=np.float32),
                              np.asarray(b2, dtype=np.float32))
    in_maps = [{"x": xh[c * B:(c + 1) * B], "bank": bankh, "consts": consts}
               for c in range(N_CORES)]
    res = run_bass_kernel_spmd(nc, in_maps, list(range(N_CORES)))
    out = np.concatenate(
        [res.results[c]["y"].reshape(B, C, H, W_IMG) for c in range(N_CORES)],
        axis=0)
    return out.astype(np.float32)


# revision 9
# speedup vs baseline: 1.1529x; 1.1529x over previous
"""DynamicConvolution Trainium2 kernel (8 NeuronCores, data-parallel over batch).

Reference computation (per sample b):
  pooled = mean(x[b], spatial); h = relu(pooled @ w1 + b1)
  alpha  = softmax(h @ w2 + b2)                   [8]
  W[b]   = sum_k alpha[k] * kernels[k]            [256,256,3,3]
  y[b]   = conv2d(x[b], W[b], pad=1)              [256,56,56]

Sharding: batch 16 -> 2 samples per core; bank + MLP weights replicated.

Device mapping (per core, all bf16 compute with fp32 PSUM accumulation):
  - x arrives host-padded as [2,256,58*58] bf16 -> contiguous DMAs into SBUF.
  - pooled via chunked free-dim reduces on VectorE (pads are zero).
  - attention MLP on TensorE with channels on partitions (no transposes);
    biases enter as K=1 rank-1 matmuls; softmax on VectorE/ScalarE.
  - alpha -> alphaT_tiled[128,2] (alphaT[c*8+k,b]=alpha[b,k]) via a K=16
    selection matmul; 16 masked lhsT tiles [128,128] built by VectorE.
  - kernel mixing ON TensorE: per (b, i_tile, o_tile, tap-group) PSUM block
    [128i, g*128] accumulates 8 matmuls; matmul j uses lhsT_j[b] (nonzero
    alpha at ((c,k), 16j+c)) against the (c,k)-partition bank layout, so 16
    i-rows mix per streamed column.  VectorE evacuates straight into the conv
    weight layout [i128, (s,o)] bf16.
  - conv: per (o_t, b, t) PSUM group of 18 accumulating matmuls
    [o128,448] += W[i128,o128]^T @ xpad[i128, 8x56 window]; ScalarE
    evacuates fp32; DMA out.

Sync discipline (walrus permits ONE semaphore wait per engine instruction):
  - matmul waits split: lhsT dep on InstLdweights, rhs dep on InstMatmult;
    PSUM-reuse WAR rides InstMatmult, so every matmul's rhs DMA queue is
    pre-observed by a 1x1 "touch" matmul, ordered with add_dep_helper.
  - mix-block PSUM evac on VectorE (conv LW waits merge on its sem);
    conv PSUM evac on ScalarE (conv MM WAR waits on its sem).
"""

import numpy as np
import ml_dtypes
from contextlib import ExitStack

try:
    import concourse.bass as bass
except ImportError:  # fresh grading dir: repo paths not on sys.path yet
    import sys
    for p in ("/opt/trn_rl_repo", "/root/.axon_site/_ro/trn_rl_repo"):
        if p not in sys.path:
            sys.path.append(p)
    import concourse.bass as bass

import concourse.mybir as mybir
import concourse.tile as tile
from concourse import bacc
from concourse.tile import add_dep_helper
from concourse.bass_utils import run_bass_kernel_spmd

F32 = mybir.dt.float32
BF16 = mybir.dt.bfloat16
AX = mybir.AxisListType.X
RELU = mybir.ActivationFunctionType.Relu
EXP = mybir.ActivationFunctionType.Exp
COPY = mybir.ActivationFunctionType.Copy

N_CORES = 8
B = 2               # samples per core
C = 256             # channels
IT = 2              # 128-channel input tiles
OT = 2              # 128-channel output tiles
H = W_IMG = 56
HW = H * W_IMG      # 3136
PADW = 58
PADHW = PADW * PADW  # 3364
NT = 7              # row blocks per image
TB = 448            # 8 rows x 56 cols per conv psum block
S = 9               # conv taps
WSL = S * 128       # 1152 = per (b,o_t,i_t) weight-slice elems
XCH = 4             # x DMA chunks per (b, i_tile)
XC = PADHW // XCH   # 841 elems per x chunk
SGRP = [(0, 4), (4, 4), (8, 1)]   # tap groups (start, len)

# mlp consts layout (fp32 [128, 336])
C_W1A, C_W1B, C_W2, C_B1, C_B2, C_ONES = 0, 64, 128, 136, 200, 208
C_COLS = 336
# mix consts layout (bf16 [128, 1154]): 8 masks | KSEL | bmask
M_MASK, M_KSEL, M_BMASK = 0, 1024, 1152
M_COLS = 1154

_cached = None


def _build():
    nc = bacc.Bacc()
    xin = nc.declare_dram_parameter("x", [B, C, PADHW], BF16, isOutput=False)
    bankin = nc.declare_dram_parameter("bank", [128, OT * IT * 8 * WSL], BF16,
                                       isOutput=False)
    cin = nc.declare_dram_parameter("consts", [128, C_COLS], F32, isOutput=False)
    min_ = nc.declare_dram_parameter("mconsts", [128, M_COLS], BF16,
                                     isOutput=False)
    y = nc.declare_dram_parameter("y", [B, C, HW], F32, isOutput=True)

    with tile.TileContext(nc) as tc, ExitStack() as ctx:
        sb = ctx.enter_context(tc.tile_pool(name="sb", bufs=1))
        conv_ps = ctx.enter_context(tc.tile_pool(name="cps", bufs=3, space="PSUM"))
        mix_ps = ctx.enter_context(tc.tile_pool(name="xps", bufs=2, space="PSUM"))
        mlp_ps = ctx.enter_context(tc.tile_pool(name="mps", bufs=1, space="PSUM"))
        scr_ps = ctx.enter_context(tc.tile_pool(name="sps", bufs=1, space="PSUM"))

        xpad = sb.tile([128, B * IT * PADHW], BF16, tag="xpad")
        bank = sb.tile([128, OT * IT * 8 * WSL], BF16, tag="bank")
        wsb = sb.tile([128, B * OT * IT * WSL], BF16, tag="wsb")
        outsb = sb.tile([128, OT * B * NT * TB], F32, tag="outsb")
        consts = sb.tile([128, C_COLS], F32, tag="consts")
        mconsts = sb.tile([128, M_COLS], BF16, tag="mconsts")
        scratch = scr_ps.tile([1, 1], F32)

        def xv(b, it):
            base = (b * IT + it) * PADHW
            return xpad[:, base:base + PADHW].rearrange("p (r c) -> p r c", c=PADW)

        # ---------- DMAs in: consts, mconsts, x (16 chunks), bank (16 slices)
        nc.sync.dma_start(consts[:], cin[:])
        nc.sync.dma_start(mconsts[:], min_[:])
        for b in range(B):
            for it in range(IT):
                base = (b * IT + it) * PADHW
                for cch in range(XCH):
                    nc.sync.dma_start(
                        xpad[:, base + cch * XC: base + (cch + 1) * XC],
                        xin[b, it * 128:(it + 1) * 128,
                            cch * XC:(cch + 1) * XC])
        # bank slices: (ot, it, j-pair) of [128, 2304] each
        for ot in range(OT):
            for it in range(IT):
                for jp in range(4):
                    off = (((ot * IT + it) * 8) + 2 * jp) * WSL
                    nc.sync.dma_start(bank[:, off:off + 2 * WSL],
                                      bankin[:, off:off + 2 * WSL])

        # ---------- PE touches for x chunks (pre-observe DMA queues)
        def pe_touch(ap):
            return nc.tensor.matmul(scratch[:], ap, ap, start=True, stop=True,
                                    skip_group_check=True)

        xtouch = []
        for b in range(B):
            for it in range(IT):
                base = (b * IT + it) * PADHW
                for cch in range(XCH):
                    xtouch.append(pe_touch(
                        xpad[0:1, base + cch * XC: base + cch * XC + 1]))
        for t1, t0 in zip(xtouch[1:], xtouch[:-1]):
            add_dep_helper(t1.ins, t0.ins, sync=False, reason="touch chain")

        # ---------- pooled means on DVE (chunked; pads are zero)
        partials = sb.tile([128, B * IT * XCH], F32, tag="partials")
        pooled = sb.tile([128, IT * B], F32, tag="pooled")  # cols (it, b)
        for b in range(B):
            for it in range(IT):
                base = (b * IT + it) * PADHW
                for cch in range(XCH):
                    j = (b * IT + it) * XCH + cch
                    nc.vector.reduce_sum(
                        partials[:, j:j + 1],
                        xpad[:, base + cch * XC: base + (cch + 1) * XC], axis=AX)
        psum2 = sb.tile([128, B * IT], F32, tag="psum2")
        for b in range(B):
            for it in range(IT):
                j = b * IT + it
                nc.vector.reduce_sum(psum2[:, j:j + 1],
                                     partials[:, j * XCH:(j + 1) * XCH], axis=AX)
                nc.vector.tensor_scalar_mul(pooled[:, it * B + b: it * B + b + 1],
                                            psum2[:, j:j + 1], 1.0 / HW)

        # ---------- attention MLP on PE
        hT_ps = mlp_ps.tile([64, B], F32, tag="hT")
        nc.tensor.matmul(hT_ps[:], consts[0:1, C_B1:C_B1 + 64],
                         consts[0:1, C_ONES:C_ONES + B], start=True, stop=False)
        nc.tensor.matmul(hT_ps[:], consts[:, C_W1A:C_W1A + 64],
                         pooled[:, 0:B], start=False, stop=False)
        nc.tensor.matmul(hT_ps[:], consts[:, C_W1B:C_W1B + 64],
                         pooled[:, B:2 * B], start=False, stop=True)
        hT = sb.tile([64, B], F32, tag="hTs")
        nc.scalar.activation(hT[:], hT_ps[:], RELU)

        sc_ps = mlp_ps.tile([B, 8], F32, tag="scps")
        nc.tensor.matmul(sc_ps[:], consts[0:1, C_ONES:C_ONES + B],
                         consts[0:1, C_B2:C_B2 + 8], start=True, stop=False)
        nc.tensor.matmul(sc_ps[:], hT[:], consts[0:64, C_W2:C_W2 + 8],
                         start=False, stop=True)

        # ---------- softmax (DVE/ACT)
        scores = sb.tile([B, 8], F32, tag="scores")
        nc.vector.tensor_copy(scores[:], sc_ps[:])
        mx = sb.tile([B, 1], F32, tag="mx")
        nc.vector.reduce_max(mx[:], scores[:], axis=AX)
        subb = sb.tile([B, 8], F32, tag="subb")
        nc.vector.tensor_scalar_sub(subb[:], scores[:], mx[:])
        ex = sb.tile([B, 8], F32, tag="ex")
        nc.scalar.activation(ex[:], subb[:], EXP)
        z = sb.tile([B, 1], F32, tag="z")
        nc.vector.reduce_sum(z[:], ex[:], axis=AX)
        rz = sb.tile([B, 1], F32, tag="rz")
        nc.vector.reciprocal(rz[:], z[:])
        al = sb.tile([B, 8], F32, tag="al")
        nc.vector.tensor_scalar_mul(al[:], ex[:], rz[:])

        # ---------- alphaT_tiled[128,2] and 16 masked lhsT tiles
        mtch = sb.tile([1, 1], BF16, tag="mtch")
        mtch_i = nc.vector.tensor_copy(mtch[:], mconsts[0:1, 0:1])

        a16 = sb.tile([16, 1], F32, tag="a16")
        nc.sync.dma_start(a16[:], al[:])            # [2,8] -> [16,1], q=b*8+k
        a2 = sb.tile([16, B], BF16, tag="a2")
        a2_i = nc.vector.tensor_scalar_mul(a2[:], mconsts[0:16, M_BMASK:M_BMASK + B],
                                           a16[:])
        add_dep_helper(a2_i.ins, mtch_i.ins, sync=False, reason="mconsts seen")
        aT_ps = mlp_ps.tile([128, B], F32, tag="scps")   # reuse scores bank
        nc.tensor.matmul(aT_ps[:], mconsts[0:16, M_KSEL:M_KSEL + 128], a2[:],
                         start=True, stop=True)
        aT = sb.tile([128, B], F32, tag="aT")
        nc.vector.tensor_copy(aT[:], aT_ps[:])
        lhsT = sb.tile([128, 16 * 128], BF16, tag="lhsT")  # (j, b) tiles
        for j in range(8):
            for b in range(B):
                nc.vector.tensor_scalar_mul(
                    lhsT[:, (j * B + b) * 128:(j * B + b + 1) * 128],
                    mconsts[:, M_MASK + j * 128:M_MASK + (j + 1) * 128],
                    aT[:, b:b + 1])

        # ---------- per o_tile: bank touches, PE mixing, conv
        for ot in range(OT):
            btch = []
            for it in range(IT):
                for jp in range(4):
                    off = (((ot * IT + it) * 8) + 2 * jp) * WSL
                    btch.append(pe_touch(bank[0:1, off:off + 1]))
            for t1, t0 in zip(btch[1:], btch[:-1]):
                add_dep_helper(t1.ins, t0.ins, sync=False, reason="btouch chain")
            if ot == 0:
                add_dep_helper(btch[0].ins, xtouch[-1].ins, sync=False,
                               reason="after x touches")

            # mixing: blocks (b, it, grp); 8 accumulated matmuls each
            for b in range(B):
                for it in range(IT):
                    woff = ((b * OT + ot) * IT + it) * WSL
                    for (s0, g) in SGRP:
                        mps = mix_ps.tile([128, 512], F32, tag="mix")
                        n = g * 128
                        first = None
                        for j in range(8):
                            boff = ((ot * IT + it) * 8 + j) * WSL + s0 * 128
                            mm = nc.tensor.matmul(
                                mps[:, 0:n],
                                lhsT[:, (j * B + b) * 128:(j * B + b + 1) * 128],
                                bank[:, boff:boff + n],
                                start=(j == 0), stop=(j == 7))
                            if first is None:
                                first = mm
                                add_dep_helper(mm.ins, btch[-1].ins, sync=False,
                                               reason="bank observed")
                        nc.vector.tensor_copy(
                            wsb[:, woff + s0 * 128: woff + (s0 + g) * 128],
                            mps[:, 0:n])

            # conv groups (b, t): 18 accumulated matmuls each
            for b in range(B):
                for t in range(NT):
                    ps = conv_ps.tile([128, TB], F32, tag="convps")
                    n_mm = 0
                    for it in range(IT):
                        woff = ((b * OT + ot) * IT + it) * WSL
                        v = xv(b, it)
                        for s in range(S):
                            kh, kw = s // 3, s % 3
                            nc.tensor.matmul(
                                ps[:],
                                wsb[:, woff + s * 128: woff + (s + 1) * 128],
                                v[:, 8 * t + kh: 8 * t + kh + 8, kw:kw + 56],
                                start=(n_mm == 0), stop=(n_mm == 17))
                            n_mm += 1
                    blk = (ot * B + b) * NT + t
                    nc.scalar.activation(outsb[:, blk * TB:(blk + 1) * TB],
                                         ps[:], COPY)
                    nc.sync.dma_start(
                        y[b, ot * 128:(ot + 1) * 128, t * TB:(t + 1) * TB],
                        outsb[:, blk * TB:(blk + 1) * TB])

    nc.compile()
    return nc


def _prep(x, kernels, w1, b1, w2, b2):
    """Host-side marshaling: dtype casts + layout/padding rearrangement only."""
    xp = np.zeros((16, C, PADW, PADW), dtype=ml_dtypes.bfloat16)
    xp[:, :, 1:57, 1:57] = x
    xh = np.ascontiguousarray(xp.reshape(16, C, PADHW))
    # bank[(c,k), (ot, it, j, s, o)] = kernels[k, ot*128+o, it*128+16j+c, s]
    kr = kernels.reshape(8, OT, 128, IT, 8, 16, S)   # k,ot,o,it,j,c,s
    bankh = np.ascontiguousarray(
        kr.transpose(5, 0, 1, 3, 4, 6, 2).reshape(128, OT * IT * 8 * WSL)
        .astype(ml_dtypes.bfloat16))
    consts = np.zeros((128, C_COLS), dtype=np.float32)
    consts[:, C_W1A:C_W1A + 64] = w1[0:128]
    consts[:, C_W1B:C_W1B + 64] = w1[128:256]
    consts[0:64, C_W2:C_W2 + 8] = w2
    consts[0, C_B1:C_B1 + 64] = b1
    consts[0, C_B2:C_B2 + 8] = b2
    consts[0, C_ONES:C_ONES + 128] = 1.0
    mc = np.zeros((128, M_COLS), dtype=ml_dtypes.bfloat16)
    r = np.arange(128)
    for j in range(8):
        m = np.zeros((128, 128), dtype=np.float32)
        m[r, 16 * j + r // 8] = 1.0
        mc[:, M_MASK + j * 128:M_MASK + (j + 1) * 128] = m
    q = np.arange(16)
    ks = np.zeros((16, 128), dtype=np.float32)
    ks[q[:, None] % 8 == np.arange(128)[None, :] % 8] = 1.0
    mc[0:16, M_KSEL:M_KSEL + 128] = ks
    bm = np.zeros((16, B), dtype=np.float32)
    bm[q // 8 == 0, 0] = 1.0
    bm[q // 8 == 1, 1] = 1.0
    mc[0:16, M_BMASK:M_BMASK + B] = bm
    return xh, bankh, consts, mc


def kernel(x, kernels, w1, b1, w2, b2):
    global _cached
    if _cached is None:
        _cached = _build()
    nc = _cached
    xh, bankh, consts, mc = _prep(np.asarray(x, dtype=np.float32),
                                  np.asarray(kernels, dtype=np.float32),
                                  np.asarray(w1, dtype=np.float32),
                                  np.asarray(b1, dtype=np.float32),
                                  np.asarray(w2, dtype=np.float32),
                                  np.asarray(b2, dtype=np.float32))
    in_maps = [{"x": xh[c * B:(c + 1) * B], "bank": bankh,
                "consts": consts, "mconsts": mc} for c in range(N_CORES)]
    res = run_bass_kernel_spmd(nc, in_maps, list(range(N_CORES)))
    out = np.concatenate(
        [res.results[c]["y"].reshape(B, C, H, W_IMG) for c in range(N_CORES)],
        axis=0)
    return out.astype(np.float32)
